# revision 45
# baseline (speedup 1.0000x reference)
"""Multi-head attention (B=4, T=2048, C=1024, H=16) on 8 trn2 NeuronCores.

Sharding: core c = 2*b + g handles batch b (of 4) and head-group g (of 2,
8 heads = 512 channels each). Each core computes q/k/v projections for its
512 channels, full TxT attention for its 8 heads, and the partial output
projection out_part = y_local @ Wo[:, g*512:(g+1)*512].T. Host sums the two
partials per batch and adds bo.

Mask trick: keys with mask!=0 contribute exactly 0 to softmax (exp(-inf)),
so the host compacts the key axis to the unmasked positions before the K/V
projections (~halves attention work). The compacted key count is padded to a
multiple of 128; padding lanes get a -1e30 bias fused into the exp.

On-chip layouts (per core):
  xT   [C=1024, T=2048]      x[b][order].T, order = kept keys first; the
                             K/V projections read columns [0:TKP)
  qp2  [128, 2048] x4        q.T packed: tile m holds heads 2m (part 0-63)
                             and 2m+1 (part 64-127)
  kT   [128, TKP] x4         k.T, same head packing
  vaug [TKP, 8*128]          per head 128 cols: 64 ones then 64 v data
  yT   [128, 2048] x4        normalized attention output transposed

Attention runs per head-PAIR: the two heads' score matmuls have K=64
contraction and execute on independent 64-row PE tiles (tile_position
(0,0) / (64,0)) concurrently, writing adjacent halves of one [128,1024]
psum tile that a single exp activation drains. The 64 ones-columns of
vaug replicate the softmax denominator on psum partitions 0-63, so
normalization is reciprocal+multiply straight from psum (no copy, no
partition broadcast).

Schedule: the scalar engine's exp stream (144 x ~1.1us, ~160us total) is the
steady-state governor; total tensor-engine streaming (~178us) sits just
below it, and HBM (8.6MB in + 4MB out at ~350GB/s) gates the start. So the
kernel starts the exp stream as early as possible - only k-pair 0's first
256 key columns and q chunk 0 are projected up front (~2.3MB of critical
DMA) - and every other projection (all of V, the rest of K/Q, the output
rows) is cut into ~1k-cycle generator quanta. The attention loop pumps
quanta between score groups against an emitted-work ledger (pe_t vs exp_t)
so the in-order PE queue always has work but never delays the next score
matmul; PV matmuls trail the exp stream by a few tiles (pend list).
Emission-order safety: Tile derives dependencies from emission order, so
ensure_* force-drains the need-ordered generator queue up to a required
producer before any consumer is emitted.
"""

import numpy as np
import ml_dtypes

import concourse.bass as bass
import concourse.mybir as mybir
import concourse.tile as tile
from concourse import bacc
from concourse.bass_utils import run_bass_kernel_spmd

F32 = mybir.dt.float32
BF16 = mybir.dt.bfloat16
NP_BF16 = ml_dtypes.bfloat16

B, T, C = 4, 2048, 1024
H, D = 16, 64
G = 2                 # head groups (cores per batch)
HL = H // G           # heads per core = 8
DL = HL * D           # local channels = 512
NP = HL // 2          # head pairs per core = 4
SCALE = 1.0 / np.sqrt(D)
NEG = -1e30
N_CORES = 8

EXP_CYC = 2671        # one [128,1024] exp, in 2.4GHz PE-cycle units

_nc_cache: dict = {}


def _dedup_ldweights(nc):
    """Drop Ldweights whose stationary operand, tile size/position and perf
    mode are identical to the immediately preceding (kept) Ldweights — the PE
    array retains its weights across matmuls, so the repeat load is pure
    overhead. Any semaphore waits on a dropped load move to the next matmul."""
    n_rm = 0
    for blk in nc.main_func.blocks:
        insts = blk.instructions
        last_key = None
        pend_waits = []
        drop = []
        for idx in range(len(insts)):
            inst = insts[idx]
            nm = type(inst).__name__
            if nm == "InstLdweights":
                a = inst.ins[0]
                key = (
                    str(getattr(a, "memref", None)),
                    str(getattr(a, "memsetref", None)),
                    a.offset, str(a.ap), str(a.dtype),
                    str(getattr(inst, "tile_size", None)),
                    str(getattr(inst, "tile_position", None)),
                    str(getattr(inst, "perf_mode", None)),
                    str(getattr(inst, "is_transpose", None)),
                )
                si = inst.sync_info
                has_upd = si is not None and len(si.on_update) > 0
                if key == last_key and not has_upd:
                    drop.append(idx)
                    if si is not None and len(si.on_wait) > 0:
                        pend_waits.extend(list(si.on_wait))
                    continue
                last_key = key
            elif nm == "InstMatmult" and pend_waits:
                si = inst.sync_info
                if si is None:
                    inst.sync_info = mybir.SyncInfo(
                        on_wait=list(pend_waits), on_update=[])
                else:
                    si.on_wait = list(si.on_wait) + list(pend_waits)
                pend_waits = []
        for idx in reversed(drop):
            del insts[idx]
        n_rm += len(drop)
    return n_rm


def _build_nc(tkp: int):
    """Build + compile the SPMD Bass program for padded key count tkp."""
    ntk = tkp // 128          # key partition-tiles
    nkc = C // 128            # contraction tiles over C = 8
    nmq = DL // 128           # channel partition-tiles = 4 (== head pairs)
    assert tkp % 128 == 0

    nc = bacc.Bacc(None, num_swdge_queues=2)

    xT_d = nc.dram_tensor("xT", [C, T], BF16, kind="ExternalInput")
    # wq/wk ship pre-swizzled to the on-chip m-major layout ([128 c-within-
    # chunk, m-block x k-chunk x 128 out]) so pair 0's weight block is one
    # small contiguous DMA on the critical path
    wqT_d = nc.dram_tensor("wqT", [128, nkc * DL], BF16, kind="ExternalInput")
    wkT_d = nc.dram_tensor("wkT", [128, nkc * DL], BF16, kind="ExternalInput")
    wvT_d = nc.dram_tensor("wvT", [C, DL], BF16, kind="ExternalInput")
    woT_d = nc.dram_tensor("woT", [DL, C], BF16, kind="ExternalInput")
    # bias_all packs [bqp | bkp | bvp | mbp] along the free dim
    nbias = nmq + nmq + DL + ntk
    bias_d = nc.dram_tensor("bias_all", [128, nbias], F32, kind="ExternalInput")
    out_d = nc.dram_tensor("out", [T, C], mybir.dt.float16, kind="ExternalOutput")

    with tile.TileContext(nc) as tc:
        with (
            tc.tile_pool(name="persist", bufs=1) as pp,
            tc.tile_pool(name="work", bufs=4) as wp,
            tc.tile_pool(name="psum", bufs=1, space="PSUM") as psp,
        ):
            # ---- persistent SBUF tensors (wide layout: k-tile k at column k*W) ----
            def persist(shape, dt, tag):
                return pp.tile(shape, dt, tag=tag, name=tag)

            xT_a = persist([128, nkc * T], BF16, "xTa")
            wqT_a = persist([128, nkc * DL], BF16, "wqTa")
            wkT_a = persist([128, nkc * DL], BF16, "wkTa")
            wvT_a = persist([128, nkc * DL], BF16, "wvTa")
            woT_a = persist([128, nmq * C], BF16, "woTa")
            qp_t = [persist([128, T], BF16, f"qp{m}") for m in range(nmq)]
            kT_t = [persist([128, tkp], BF16, f"kT{m}") for m in range(nmq)]
            va_t = [persist([128, HL * 128], BF16, f"va{t}") for t in range(ntk)]
            yT_t = [persist([128, T], BF16, f"yT{m}") for m in range(nmq)]
            bias_t = persist([128, nbias], F32, "bias")
            OQ, OK, OV, OM = 0, nmq, 2 * nmq, 2 * nmq + DL
            scr_t = persist([128, 640], BF16, "scr")  # PE warmup scratch

            # psum slots (8 banks): "s" 2x[128,1024] (4), "y" 2x[128,512] (2),
            # "f" 2x[128,512] (2). During attention, fillers use only "f".
            def psum_tile(shape, tag, name):
                return psp.tile(shape, F32, tag=tag, name=name, bufs=2)

            # ---- input DMAs, in global need-order, round-robin over the 3
            # trigger queues (2 HW-DGE + gpsimd SW-DGE). HBM is the startup
            # bottleneck (~350GB/s aggregate), so the first-exp critical set
            # (bias, wk/wq pair-0 blocks, x front columns 0:640) goes first.
            QENG = [nc.sync, nc.gpsimd, nc.scalar]
            qi = [0]

            def rrq():
                e = QENG[qi[0] % len(QENG)]
                qi[0] += 1
                return e

            def dma(sb, dram, W, k0, k1, eng):
                src = dram[:].rearrange("(k p) n -> p k n", p=128)[:, k0:k1, :]
                dst = sb[:, k0 * W:k1 * W].rearrange("p (k n) -> p k n", n=W)
                eng.dma_start(out=dst, in_=src)

            def dma_cols(sb, dram, W, k, c0, c1, eng):
                src = dram[:].rearrange("(k p) n -> p k n", p=128)[:, k:k + 1, c0:c1]
                dst = sb[:, k * W + c0:k * W + c1].rearrange(
                    "p (k n) -> p k n", n=c1 - c0)
                eng.dma_start(out=dst, in_=src)

            MB = nkc * 128    # columns per m-block in the wq/wk layout
            FR = min(512, tkp)  # critical x slice: feeds k0[0:256] and q00
            nc.scalar.dma_start(out=bias_t[:], in_=bias_d[:])
            rrq().dma_start(out=wkT_a[:, 0:MB], in_=wkT_d[:, 0:MB])
            rrq().dma_start(out=wqT_a[:, 0:MB], in_=wqT_d[:, 0:MB])
            for k in range(nkc):
                dma_cols(xT_a, xT_d, T, k, 0, FR, rrq())
            for k in range(nkc):
                dma(wvT_a, wvT_d, DL, k, k + 1, rrq())
            if FR < tkp:
                for k in range(nkc):
                    dma_cols(xT_a, xT_d, T, k, FR, tkp, rrq())
            rrq().dma_start(out=wkT_a[:, MB:nmq * MB], in_=wkT_d[:, MB:nmq * MB])
            rrq().dma_start(out=wqT_a[:, MB:nmq * MB], in_=wqT_d[:, MB:nmq * MB])
            if tkp < T:
                for k in range(nkc):
                    dma_cols(xT_a, xT_d, T, k, tkp, T, rrq())
            dma(woT_a, woT_d, C, 0, nmq // 2, rrq())
            dma(woT_a, woT_d, C, nmq // 2, nmq, rrq())

            # ---- PE warmup: trip the HAM clock gate while DMA streams in ----
            nc.vector.memset(scr_t[:], 0.0)
            wps = psp.tile([128, 256], F32, tag="s", name="warmup", bufs=2)
            for w in range(6):
                nc.tensor.matmul(
                    wps[:], lhsT=scr_t[:, 0:128], rhs=scr_t[:, 128:384],
                    start=(w == 0), stop=(w == 5),
                )

            # va ones columns (only cols 0:64 of each head block need init)
            for t in range(ntk):
                nc.vector.memset(
                    va_t[t][:].rearrange("p (h e) -> p h e", e=128)[:, :, 0:64], 1.0)

            uid = [0]
            bv3 = bias_t[:, OV:OV + DL].rearrange("p (h e) -> p h e", e=D)

            # readiness markers for emission-order safety
            va_done = set()
            kcov = {m: 0 for m in range(nmq)}   # kT_t[m] columns written
            q_done = set()

            # ---- startup: only the minimal k/q blocks the first score
            # matmul needs; every V tile is a paced filler (the PV queue
            # defers their consumers past the first attentions) ----
            # k pair 0, key columns 0:256 (chunk-major; LDW per matmul)
            psk = psum_tile([128, 256], "y", "kps0_narrow")
            for k in range(nkc):
                nc.tensor.matmul(
                    psk[:], lhsT=wkT_a[:, k * 128:(k + 1) * 128],
                    rhs=xT_a[:, k * T:k * T + 256],
                    start=(k == 0), stop=(k == nkc - 1),
                )
            nc.vector.tensor_scalar_add(
                kT_t[0][:, 0:256], psk[:], bias_t[:, OK:OK + 1])
            kcov[0] = 256
            # q pair 0, chunk 0
            psq = psum_tile([128, 512], "s", "qps00")
            for k in range(nkc):
                nc.tensor.matmul(
                    psq[:], lhsT=wqT_a[:, k * 128:(k + 1) * 128],
                    rhs=xT_a[:, k * T:k * T + 512],
                    start=(k == 0), stop=(k == nkc - 1),
                )
            nc.vector.tensor_scalar_add(
                qp_t[0][:, 0:512], psq[:], bias_t[:, OQ:OQ + 1])
            q_done.add((0, 0))

            # ---- filler generators: each yield returns its PE-cycle cost ----
            def gen_v_tile(t, tag="f"):
                uid[0] += 1
                ps = psum_tile([128, DL], tag, f"vps{t}_{uid[0]}")
                for k0 in range(0, nkc, 2):
                    for k in (k0, k0 + 1):
                        nc.tensor.matmul(
                            ps[:],
                            lhsT=xT_a[:, k * T + t * 128:k * T + (t + 1) * 128],
                            rhs=wvT_a[:, k * DL:(k + 1) * DL],
                            start=(k == 0), stop=(k == nkc - 1),
                        )
                    yield 2 * DL
                dst = va_t[t][:].rearrange("p (h e) -> p h e", e=128)[:, :, 64:128]
                src = ps[:].rearrange("p (h e) -> p h e", e=D)
                nc.vector.tensor_add(dst, src, bv3)
                va_done.add(t)
                yield 64

            def gen_k_group(m, chunks, tag="f"):
                uid[0] += 1
                pss = [psum_tile([128, 512], tag, f"kps{uid[0]}_{s0}")
                       for s0, cn in chunks]
                for k in range(nkc):
                    lhsT = wkT_a[:, (m * nkc + k) * 128:(m * nkc + k + 1) * 128]
                    for ps, (s0, cn) in zip(pss, chunks):
                        nc.tensor.matmul(
                            ps[:, 0:cn], lhsT=lhsT,
                            rhs=xT_a[:, k * T + s0:k * T + s0 + cn],
                            start=(k == 0), stop=(k == nkc - 1),
                        )
                    yield sum(cn for _, cn in chunks)
                for ps, (s0, cn) in zip(pss, chunks):
                    nc.vector.tensor_scalar_add(
                        kT_t[m][:, s0:s0 + cn], ps[:, 0:cn],
                        bias_t[:, OK + m:OK + m + 1])
                kcov[m] = max(kcov[m], max(s0 + cn for s0, cn in chunks))
                yield 64

            def gen_q_group(m, ns, tag="f"):
                uid[0] += 1
                pss = [psum_tile([128, 512], tag, f"qps{uid[0]}_{n}")
                       for n in ns]
                for k in range(nkc):
                    lhsT = wqT_a[:, (m * nkc + k) * 128:(m * nkc + k + 1) * 128]
                    for ps, n in zip(pss, ns):
                        nc.tensor.matmul(
                            ps[:], lhsT=lhsT,
                            rhs=xT_a[:, k * T + n * 512:k * T + (n + 1) * 512],
                            start=(k == 0), stop=(k == nkc - 1),
                        )
                    yield 512 * len(ns)
                for ps, n in zip(pss, ns):
                    nc.vector.tensor_scalar_add(
                        qp_t[m][:, n * 512:(n + 1) * 512], ps[:],
                        bias_t[:, OQ + m:OQ + m + 1])
                q_done.update((m, n) for n in ns)
                yield 64

            OENG = [nc.sync, nc.gpsimd]
            oq = [0]

            o_state = {}

            def o_quanta(mt, tag="f", kts=None):
                """Output-projection rows mt*128: list of (kind, cost, fn)
                quanta for the pv queue (FIFO after the norms that write its
                yT). kts selects a subset of the accumulation steps so the
                final rows can pre-run all but their last yT dependency."""
                state = o_state.setdefault(mt, {})
                kts = range(nmq) if kts is None else kts
                last = max(kts) == nmq - 1

                def mm_q(kt):
                    def fn():
                        if "pss" not in state:
                            uid[0] += 1
                            state["pss"] = [
                                psum_tile([128, 512], tag, f"ops{uid[0]}_{h2}")
                                for h2 in range(2)]
                        lhsT = yT_t[kt][:, mt * 128:(mt + 1) * 128]
                        for h2 in range(2):
                            nc.tensor.matmul(
                                state["pss"][h2][:], lhsT=lhsT,
                                rhs=woT_a[:, kt * C + h2 * 512:
                                          kt * C + (h2 + 1) * 512],
                                start=(kt == 0), stop=(kt == nmq - 1),
                            )
                    return ("o", 1024, fn)

                def out_q(h2):
                    def fn():
                        uid[0] += 1
                        o_sb = wp.tile([128, 512], mybir.dt.float16, tag="o",
                                       name=f"osb{uid[0]}_{h2}", bufs=4)
                        nc.vector.tensor_copy(o_sb[:], state["pss"][h2][:])
                        eng = OENG[oq[0] % 2]
                        oq[0] += 1
                        eng.dma_start(
                            out=out_d[mt * 128:(mt + 1) * 128,
                                      h2 * 512:(h2 + 1) * 512],
                            in_=o_sb[:])
                    return ("o", 64, fn)

                qs = [mm_q(kt) for kt in kts]
                if last:
                    qs += [out_q(0), out_q(1)]
                return qs

            # ---- filler queue in need-order ----
            # k0 remainder chunks, split at the front boundary FR
            k0_rest = [(256, min(FR, tkp) - 256)] if tkp > 256 else []
            k0_rest += [(s0, min(512, tkp - s0)) for s0 in range(FR, tkp, 512)]
            k_chunks = [(s0, min(512, tkp - s0)) for s0 in range(0, tkp, 512)]
            k_groups = [k_chunks[i:i + 2] for i in range(0, len(k_chunks), 2)]

            # need-order: k0 remainder interleaved with the first V tiles
            # (scores t2+ need kT columns before PVs need va), q chunk 1
            # after ~6 V tiles, then per-pair k+q for the later attentions
            gens = []
            nv = 0
            for ch in k0_rest:
                gens.append(gen_k_group(0, [ch]))
                if nv < ntk:
                    gens.append(gen_v_tile(nv))
                    nv += 1
            while nv < min(6, ntk):
                gens.append(gen_v_tile(nv))
                nv += 1
            gens.append(gen_q_group(0, [1]))
            while nv < ntk:
                gens.append(gen_v_tile(nv))
                nv += 1
            for m in range(1, nmq):
                for g in k_groups:
                    gens.append(gen_k_group(m, g))
                gens.append(gen_q_group(m, [0, 1]))

            # emitted-work ledger: pump background work only while the PE
            # queue has less emitted work than the exp stream will take to
            # execute. Two sources: `gens` (projection fillers - they gate
            # future exps, so they go first) and `pvq` (the cross-attention
            # PV/norm/output FIFO - it only gates psum slots). `cool` spaces
            # the first PV after a norm so the normalize's DVE latency never
            # stalls the in-order PE queue.
            led = {"pe": 0, "exp": 0, "cool": 0}
            pvq = []          # FIFO of (kind, cost, fn)
            npv = [0]         # count of 'pv' items in pvq (p_sb slots held)

            def force_step():
                try:
                    led["pe"] += next(gens[0])
                except StopIteration:
                    gens.pop(0)

            def pop_pvq():
                kind, cost, fn = pvq.pop(0)
                if kind == "pv":
                    npv[0] -= 1
                if kind == "norm":
                    led["cool"] = led["pe"] + 4200
                fn()
                led["pe"] += cost

            def pump():
                while led["pe"] + 500 < led["exp"]:
                    if npv[0] > 12:
                        pop_pvq()
                    elif gens:
                        force_step()
                    elif pvq and (led["pe"] >= led["cool"]
                                  or pvq[0][0] != "pv"):
                        pop_pvq()
                    else:
                        break

            def drain_pvq():
                while pvq:
                    pop_pvq()

            def ensure_va(t):
                while t not in va_done and gens:
                    force_step()

            def ensure_kq(m, t_need, qc):
                while gens and (kcov[m] < t_need or (m, qc) not in q_done):
                    force_step()

            EXPF = mybir.ActivationFunctionType.Exp

            def attention(m, qc):
                """Head pair m (heads 2m, 2m+1), query chunk qc (512 wide).
                Emits only the S matmuls and exps; PV accumulation and the
                normalize are queued on pvq and drain under the ledger."""
                q0 = qc * 512
                uid[0] += 1
                yps = psum_tile([128, 512], "y", f"yps{uid[0]}")
                yps2 = psum_tile([128, 512], "y", f"yps2_{uid[0]}")

                def pv_fn(t, p_sb):
                    def fn():
                        ensure_va(t)
                        nc.tensor.matmul(
                            yps[:],
                            lhsT=va_t[t][:, (2 * m) * 128:(2 * m + 1) * 128],
                            rhs=p_sb[:, 0:512],
                            start=(t == 0), stop=(t == ntk - 1),
                        )
                        nc.tensor.matmul(
                            yps2[:],
                            lhsT=va_t[t][:, (2 * m + 1) * 128:(2 * m + 2) * 128],
                            rhs=p_sb[:, 512:1024],
                            start=(t == 0), stop=(t == ntk - 1),
                        )
                    return fn

                def norm_fn():
                    # normalize straight from psum: partitions 0-63 hold the
                    # denominator (ones-columns), 64-127 the numerator
                    def fn():
                        uid[0] += 1
                        for hp, ps in ((0, yps), (1, yps2)):
                            rec = wp.tile([128, 512], F32, tag="rec",
                                          name=f"rec{uid[0]}_{hp}", bufs=2)
                            nc.vector.reciprocal_approx_fast(
                                rec[0:64, :], ps[0:64, :])
                            nc.vector.tensor_mul(
                                yT_t[m][64 * hp:64 * hp + 64, q0:q0 + 512],
                                ps[64:128, :], rec[0:64, :],
                            )
                    return fn

                def s_mm(t):
                    # two K=64 matmuls on independent 64-row PE tiles
                    uid[0] += 1
                    s_ps = psum_tile([128, 1024], "s", f"sps{uid[0]}")
                    nc.tensor.matmul(
                        s_ps[:, 0:512],
                        lhsT=kT_t[m][0:64, t * 128:(t + 1) * 128],
                        rhs=qp_t[m][0:64, q0:q0 + 512],
                        start=True, stop=True,
                    )
                    nc.tensor.matmul(
                        s_ps[:, 512:1024],
                        lhsT=kT_t[m][64:128, t * 128:(t + 1) * 128],
                        rhs=qp_t[m][64:128, q0:q0 + 512],
                        start=True, stop=True,
                    )
                    led["pe"] += 512
                    return s_ps

                # S matmuls batched two t-steps at a time: the 64-row loads of
                # step t+1 overlap the streaming of step t's opposite tile
                for t0 in range(0, ntk, 2):
                    ts = [t for t in (t0, t0 + 1) if t < ntk]
                    ensure_kq(m, (ts[-1] + 1) * 128, qc)
                    sps = [s_mm(t) for t in ts]
                    for t, s_ps in zip(ts, sps):
                        p_sb = wp.tile([128, 1024], BF16, tag="p",
                                       name=f"p{uid[0]}_{t}", bufs=20)
                        nc.scalar.activation(
                            p_sb[:], s_ps[:], EXPF,
                            bias=bias_t[:, OM + t:OM + t + 1], scale=float(SCALE),
                        )
                        led["exp"] += EXP_CYC
                        pvq.append(("pv", 1024, pv_fn(t, p_sb)))
                        npv[0] += 1
                    pump()
                pvq.append(("norm", 64, norm_fn()))

            # schedule: qc 0/1 interleaved per pair (halves the prerequisite
            # pressure of each pair's k+q projections), then qc 2/3 pair-major
            sched = ([(m, qc) for m in range(NP) for qc in (0, 1)]
                     + [(m, qc) for qc in (2, 3) for m in range(NP)])
            for i, (m, qc) in enumerate(sched):
                if i == 5:
                    for mm in range(nmq):
                        gens.append(gen_q_group(mm, [2, 3]))
                if i == 8:
                    # qc 0/1 norms are already queued ahead in the FIFO
                    for mt in range(0, 8):
                        pvq.extend(o_quanta(mt))
                if i == 12:
                    for mt in range(8, 12):
                        pvq.extend(o_quanta(mt))
                if i == 15:
                    # final rows 12/13: all but the last accumulation step can
                    # run during the last attention (their yT is ready)
                    for mt in (12, 13):
                        pvq.extend(o_quanta(mt, kts=[0, 1, 2]))
                attention(m, qc)

            # drain: remaining projections, the PV/norm backlog, then the
            # final output rows. Rows 14/15 accumulate their first steps on
            # the s slots (free once the last exp has read them) while the
            # final normalize runs on the vector engine.
            led["exp"] += 1 << 30
            while gens:
                force_step()
            for mt in (14, 15):
                pvq.extend(o_quanta(mt, "s", kts=[0, 1, 2]))
            for mt in (12, 13, 14, 15):
                pvq.extend(o_quanta(mt, kts=[3]))
            drain_pvq()

    _dedup_ldweights(nc)
    nc.compile()
    return nc


def _get_nc(tkp: int):
    if tkp not in _nc_cache:
        _nc_cache[tkp] = _build_nc(tkp)
    return _nc_cache[tkp]


def kernel(x, mask, Wk, bk, Wq, bq, Wv, bv, Wo, bo, _run_kwargs=None):
    x = np.asarray(x, dtype=np.float32)
    mask = np.asarray(mask)
    Wk, bk = np.asarray(Wk, np.float32), np.asarray(bk, np.float32)
    Wq, bq = np.asarray(Wq, np.float32), np.asarray(bq, np.float32)
    Wv, bv = np.asarray(Wv, np.float32), np.asarray(bv, np.float32)
    Wo, bo = np.asarray(Wo, np.float32), np.asarray(bo, np.float32)

    keep = [np.flatnonzero(mask[b] == 0) for b in range(B)]
    max_keep = max(len(kp) for kp in keep)
    tkp = max(128, -(-max_keep // 128) * 128)
    ntk = tkp // 128
    nmq = DL // 128

    nc = _get_nc(tkp)

    in_maps = []
    orders = []
    for b in range(B):
        # kept-key positions first: the device reads keys as xT[:, :tkp]
        order = np.concatenate([keep[b], np.flatnonzero(mask[b] != 0)])
        orders.append(order)
        xT = np.ascontiguousarray(x[b][order].T).astype(NP_BF16)
        mb = np.zeros(tkp, np.float32)
        mb[len(keep[b]):] = NEG
        mbp = np.ascontiguousarray(mb.reshape(ntk, 128).T)
        for g in range(G):
            gs, ge = g * DL, (g + 1) * DL
            bias_all = np.concatenate([
                bq[gs:ge].reshape(nmq, 128).T,
                bk[gs:ge].reshape(nmq, 128).T,
                np.broadcast_to(bv[gs:ge], (128, DL)),
                mbp,
            ], axis=1).astype(np.float32)

            def mswz(W):
                # [C, DL] -> [128, m-block x k-chunk x 128]: block (m, k)
                # holds W.T rows k*128:(k+1)*128, cols m*128:(m+1)*128
                A = np.ascontiguousarray(W[gs:ge].T).reshape(8, 128, nmq, 128)
                return np.ascontiguousarray(
                    A.transpose(1, 2, 0, 3).reshape(128, nmq * 8 * 128)
                ).astype(NP_BF16)

            in_maps.append({
                "xT": xT,
                "wqT": mswz(Wq),
                "wkT": mswz(Wk),
                "wvT": np.ascontiguousarray(Wv[gs:ge].T).astype(NP_BF16),
                "woT": np.ascontiguousarray(Wo[:, gs:ge].T).astype(NP_BF16),
                "bias_all": np.ascontiguousarray(bias_all),
            })

    kw = _run_kwargs or {}
    res = run_bass_kernel_spmd(nc, in_maps, list(range(N_CORES)), **kw)

    out = np.empty((B, T, C), np.float32)
    for b in range(B):
        summed = (res.results[2 * b]["out"].astype(np.float32)
                  + res.results[2 * b + 1]["out"].astype(np.float32) + bo)
        out[b][orders[b]] = summed  # undo the query-position permutation
    if kw:
        kernel.last_result = res
    return out


# revision 47
# speedup vs baseline: 1.0099x; 1.0099x over previous
"""Multi-head attention (B=4, T=2048, C=1024, H=16) on 8 trn2 NeuronCores.

Sharding: core c = 2*b + g handles batch b (of 4) and head-group g (of 2,
8 heads = 512 channels each). Each core computes q/k/v projections for its
512 channels, full TxT attention for its 8 heads, and the partial output
projection out_part = y_local @ Wo[:, g*512:(g+1)*512].T. Host sums the two
partials per batch and adds bo.

Mask trick: keys with mask!=0 contribute exactly 0 to softmax (exp(-inf)),
so the host compacts the key axis to the unmasked positions before the K/V
projections (~halves attention work). The compacted key count is padded to a
multiple of 128; padding lanes get a -1e30 bias fused into the exp.

On-chip layouts (per core):
  xT   [C=1024, T=2048]      x[b][order].T, order = kept keys first; the
                             K/V projections read columns [0:TKP)
  qp2  [128, 2048] x4        q.T packed: tile m holds heads 2m (part 0-63)
                             and 2m+1 (part 64-127)
  kT   [128, TKP] x4         k.T, same head packing
  vaug [TKP, 8*128]          per head 128 cols: 64 ones then 64 v data
  yT   [128, 2048] x4        normalized attention output transposed

Attention runs per head-PAIR: the two heads' score matmuls have K=64
contraction and execute on independent 64-row PE tiles (tile_position
(0,0) / (64,0)) concurrently, writing adjacent halves of one [128,1024]
psum tile that a single exp activation drains. The 64 ones-columns of
vaug replicate the softmax denominator on psum partitions 0-63, so
normalization is reciprocal+multiply straight from psum (no copy, no
partition broadcast).

Schedule: the scalar engine's exp stream (144 x ~1.1us, ~160us total) is the
steady-state governor; total tensor-engine streaming (~178us) sits just
below it, and HBM (8.6MB in + 4MB out at ~350GB/s) gates the start. So the
kernel starts the exp stream as early as possible - only k-pair 0's first
256 key columns and q chunk 0 are projected up front (~2.3MB of critical
DMA) - and every other projection (all of V, the rest of K/Q, the output
rows) is cut into ~1k-cycle generator quanta. The attention loop pumps
quanta between score groups against an emitted-work ledger (pe_t vs exp_t)
so the in-order PE queue always has work but never delays the next score
matmul; PV matmuls trail the exp stream by a few tiles (pend list).
Emission-order safety: Tile derives dependencies from emission order, so
ensure_* force-drains the need-ordered generator queue up to a required
producer before any consumer is emitted.
"""

import numpy as np
import ml_dtypes

import concourse.bass as bass
import concourse.mybir as mybir
import concourse.tile as tile
from concourse import bacc
from concourse.bass_utils import run_bass_kernel_spmd

F32 = mybir.dt.float32
BF16 = mybir.dt.bfloat16
NP_BF16 = ml_dtypes.bfloat16

B, T, C = 4, 2048, 1024
H, D = 16, 64
G = 2                 # head groups (cores per batch)
HL = H // G           # heads per core = 8
DL = HL * D           # local channels = 512
NP = HL // 2          # head pairs per core = 4
SCALE = 1.0 / np.sqrt(D)
NEG = -1e30
N_CORES = 8

EXP_CYC = 2671        # one [128,1024] exp, in 2.4GHz PE-cycle units

_nc_cache: dict = {}


def _dedup_ldweights(nc):
    """Drop Ldweights whose stationary operand, tile size/position and perf
    mode are identical to the immediately preceding (kept) Ldweights — the PE
    array retains its weights across matmuls, so the repeat load is pure
    overhead. Any semaphore waits on a dropped load move to the next matmul."""
    n_rm = 0
    for blk in nc.main_func.blocks:
        insts = blk.instructions
        last_key = None
        pend_waits = []
        drop = []
        for idx in range(len(insts)):
            inst = insts[idx]
            nm = type(inst).__name__
            if nm == "InstLdweights":
                a = inst.ins[0]
                key = (
                    str(getattr(a, "memref", None)),
                    str(getattr(a, "memsetref", None)),
                    a.offset, str(a.ap), str(a.dtype),
                    str(getattr(inst, "tile_size", None)),
                    str(getattr(inst, "tile_position", None)),
                    str(getattr(inst, "perf_mode", None)),
                    str(getattr(inst, "is_transpose", None)),
                )
                si = inst.sync_info
                has_upd = si is not None and len(si.on_update) > 0
                if key == last_key and not has_upd:
                    drop.append(idx)
                    if si is not None and len(si.on_wait) > 0:
                        pend_waits.extend(list(si.on_wait))
                    continue
                last_key = key
            elif nm == "InstMatmult" and pend_waits:
                si = inst.sync_info
                if si is None:
                    inst.sync_info = mybir.SyncInfo(
                        on_wait=list(pend_waits), on_update=[])
                else:
                    si.on_wait = list(si.on_wait) + list(pend_waits)
                pend_waits = []
        for idx in reversed(drop):
            del insts[idx]
        n_rm += len(drop)
    return n_rm


def _build_nc(tkp: int):
    """Build + compile the SPMD Bass program for padded key count tkp."""
    ntk = tkp // 128          # key partition-tiles
    nkc = C // 128            # contraction tiles over C = 8
    nmq = DL // 128           # channel partition-tiles = 4 (== head pairs)
    assert tkp % 128 == 0

    nc = bacc.Bacc(None, num_swdge_queues=2)

    xT_d = nc.dram_tensor("xT", [C, T], BF16, kind="ExternalInput")
    # wq/wk ship pre-swizzled to the on-chip m-major layout ([128 c-within-
    # chunk, m-block x k-chunk x 128 out]) so pair 0's weight block is one
    # small contiguous DMA on the critical path
    wqT_d = nc.dram_tensor("wqT", [128, nkc * DL], BF16, kind="ExternalInput")
    wkT_d = nc.dram_tensor("wkT", [128, nkc * DL], BF16, kind="ExternalInput")
    wvT_d = nc.dram_tensor("wvT", [C, DL], BF16, kind="ExternalInput")
    woT_d = nc.dram_tensor("woT", [DL, C], BF16, kind="ExternalInput")
    # bias_all packs [bqp | bkp | bvp | mbp] along the free dim
    nbias = nmq + nmq + DL + ntk
    bias_d = nc.dram_tensor("bias_all", [128, nbias], F32, kind="ExternalInput")
    out_d = nc.dram_tensor("out", [T, C], mybir.dt.float16, kind="ExternalOutput")

    with tile.TileContext(nc) as tc:
        with (
            tc.tile_pool(name="persist", bufs=1) as pp,
            tc.tile_pool(name="work", bufs=4) as wp,
            tc.tile_pool(name="psum", bufs=1, space="PSUM") as psp,
        ):
            # ---- persistent SBUF tensors (wide layout: k-tile k at column k*W) ----
            def persist(shape, dt, tag):
                return pp.tile(shape, dt, tag=tag, name=tag)

            xT_a = persist([128, nkc * T], BF16, "xTa")
            wqT_a = persist([128, nkc * DL], BF16, "wqTa")
            wkT_a = persist([128, nkc * DL], BF16, "wkTa")
            wvT_a = persist([128, nkc * DL], BF16, "wvTa")
            woT_a = persist([128, nmq * C], BF16, "woTa")
            qp_t = [persist([128, T], BF16, f"qp{m}") for m in range(nmq)]
            kT_t = [persist([128, tkp], BF16, f"kT{m}") for m in range(nmq)]
            va_t = [persist([128, HL * 128], BF16, f"va{t}") for t in range(ntk)]
            yT_t = [persist([128, T], BF16, f"yT{m}") for m in range(nmq)]
            bias_t = persist([128, nbias], F32, "bias")
            OQ, OK, OV, OM = 0, nmq, 2 * nmq, 2 * nmq + DL
            scr_t = persist([128, 640], BF16, "scr")  # PE warmup scratch

            # psum slots (8 banks): "s" 2x[128,1024] (4), "y" 2x[128,512] (2),
            # "f" 2x[128,512] (2). During attention, fillers use only "f".
            def psum_tile(shape, tag, name):
                return psp.tile(shape, F32, tag=tag, name=name, bufs=2)

            # ---- input DMAs, in global need-order, round-robin over the 3
            # trigger queues (2 HW-DGE + gpsimd SW-DGE). HBM is the startup
            # bottleneck (~350GB/s aggregate), so the first-exp critical set
            # (bias, wk/wq pair-0 blocks, x front columns 0:640) goes first.
            QENG = [nc.sync, nc.gpsimd, nc.scalar]
            qi = [0]

            def rrq():
                e = QENG[qi[0] % len(QENG)]
                qi[0] += 1
                return e

            def dma(sb, dram, W, k0, k1, eng):
                src = dram[:].rearrange("(k p) n -> p k n", p=128)[:, k0:k1, :]
                dst = sb[:, k0 * W:k1 * W].rearrange("p (k n) -> p k n", n=W)
                eng.dma_start(out=dst, in_=src)

            def dma_cols(sb, dram, W, k, c0, c1, eng):
                src = dram[:].rearrange("(k p) n -> p k n", p=128)[:, k:k + 1, c0:c1]
                dst = sb[:, k * W + c0:k * W + c1].rearrange(
                    "p (k n) -> p k n", n=c1 - c0)
                eng.dma_start(out=dst, in_=src)

            MB = nkc * 128    # columns per m-block in the wq/wk layout
            FR = min(512, tkp)  # critical x slice: feeds k0[0:256] and q00
            # critical stream: the DMA rings run ~4 descriptors deep per
            # queue CONCURRENTLY, so anything enqueued now shares bandwidth
            # immediately - only truly critical bytes go in up front
            nc.scalar.dma_start(out=bias_t[:], in_=bias_d[:])
            rrq().dma_start(out=wkT_a[:, 0:MB], in_=wkT_d[:, 0:MB])
            rrq().dma_start(out=wqT_a[:, 0:MB], in_=wqT_d[:, 0:MB])
            for k in range(nkc):
                dma_cols(xT_a, xT_d, T, k, 0, FR, rrq())
            if FR < tkp:
                for k in range(nkc):
                    dma_cols(xT_a, xT_d, T, k, FR, tkp, rrq())

            # ---- PE warmup: trip the HAM clock gate while DMA streams in ----
            nc.vector.memset(scr_t[:], 0.0)
            wps = psp.tile([128, 256], F32, tag="s", name="warmup", bufs=2)
            for w in range(6):
                nc.tensor.matmul(
                    wps[:], lhsT=scr_t[:, 0:128], rhs=scr_t[:, 128:384],
                    start=(w == 0), stop=(w == 5),
                )

            # va ones columns (only cols 0:64 of each head block need init)
            for t in range(ntk):
                nc.vector.memset(
                    va_t[t][:].rearrange("p (h e) -> p h e", e=128)[:, :, 0:64], 1.0)

            uid = [0]
            bv3 = bias_t[:, OV:OV + DL].rearrange("p (h e) -> p h e", e=D)

            # readiness markers for emission-order safety
            va_done = set()
            kcov = {m: 0 for m in range(nmq)}   # kT_t[m] columns written
            q_done = set()

            # ---- startup: only the minimal k/q blocks the first score
            # matmul needs; every V tile is a paced filler (the PV queue
            # defers their consumers past the first attentions) ----
            # k pair 0, key columns 0:256 (chunk-major; LDW per matmul)
            psk = psum_tile([128, 256], "y", "kps0_narrow")
            for k in range(nkc):
                nc.tensor.matmul(
                    psk[:], lhsT=wkT_a[:, k * 128:(k + 1) * 128],
                    rhs=xT_a[:, k * T:k * T + 256],
                    start=(k == 0), stop=(k == nkc - 1),
                )
            nc.vector.tensor_scalar_add(
                kT_t[0][:, 0:256], psk[:], bias_t[:, OK:OK + 1])
            kcov[0] = 256
            # gate the remaining input DMAs behind the critical stream: this
            # copy depends on the k0 add, so everything after it on gpsimd
            # waits until the critical x columns have landed
            gate_t = persist([128, 8], BF16, "gate")
            nc.gpsimd.tensor_copy(gate_t[:, 0:1], kT_t[0][:, 0:1])
            for k in range(nkc):
                dma(wvT_a, wvT_d, DL, k, k + 1, nc.gpsimd)
            nc.gpsimd.dma_start(out=wkT_a[:, MB:nmq * MB],
                                in_=wkT_d[:, MB:nmq * MB])
            nc.gpsimd.dma_start(out=wqT_a[:, MB:nmq * MB],
                                in_=wqT_d[:, MB:nmq * MB])
            if tkp < T:
                for k in range(nkc):
                    dma_cols(xT_a, xT_d, T, k, tkp, T, nc.gpsimd)
            dma(woT_a, woT_d, C, 0, nmq // 2, nc.gpsimd)
            dma(woT_a, woT_d, C, nmq // 2, nmq, nc.gpsimd)
            # q pair 0, chunk 0
            psq = psum_tile([128, 512], "s", "qps00")
            for k in range(nkc):
                nc.tensor.matmul(
                    psq[:], lhsT=wqT_a[:, k * 128:(k + 1) * 128],
                    rhs=xT_a[:, k * T:k * T + 512],
                    start=(k == 0), stop=(k == nkc - 1),
                )
            nc.vector.tensor_scalar_add(
                qp_t[0][:, 0:512], psq[:], bias_t[:, OQ:OQ + 1])
            q_done.add((0, 0))

            # ---- filler generators: each yield returns its PE-cycle cost ----
            def gen_v_tile(t, tag="f"):
                uid[0] += 1
                ps = psum_tile([128, DL], tag, f"vps{t}_{uid[0]}")
                for k0 in range(0, nkc, 2):
                    for k in (k0, k0 + 1):
                        nc.tensor.matmul(
                            ps[:],
                            lhsT=xT_a[:, k * T + t * 128:k * T + (t + 1) * 128],
                            rhs=wvT_a[:, k * DL:(k + 1) * DL],
                            start=(k == 0), stop=(k == nkc - 1),
                        )
                    yield 2 * DL
                dst = va_t[t][:].rearrange("p (h e) -> p h e", e=128)[:, :, 64:128]
                src = ps[:].rearrange("p (h e) -> p h e", e=D)
                nc.vector.tensor_add(dst, src, bv3)
                va_done.add(t)
                yield 64

            def gen_k_group(m, chunks, tag="f"):
                uid[0] += 1
                pss = [psum_tile([128, 512], tag, f"kps{uid[0]}_{s0}")
                       for s0, cn in chunks]
                for k in range(nkc):
                    lhsT = wkT_a[:, (m * nkc + k) * 128:(m * nkc + k + 1) * 128]
                    for ps, (s0, cn) in zip(pss, chunks):
                        nc.tensor.matmul(
                            ps[:, 0:cn], lhsT=lhsT,
                            rhs=xT_a[:, k * T + s0:k * T + s0 + cn],
                            start=(k == 0), stop=(k == nkc - 1),
                        )
                    yield sum(cn for _, cn in chunks)
                for ps, (s0, cn) in zip(pss, chunks):
                    nc.vector.tensor_scalar_add(
                        kT_t[m][:, s0:s0 + cn], ps[:, 0:cn],
                        bias_t[:, OK + m:OK + m + 1])
                kcov[m] = max(kcov[m], max(s0 + cn for s0, cn in chunks))
                yield 64

            def gen_q_group(m, ns, tag="f"):
                uid[0] += 1
                pss = [psum_tile([128, 512], tag, f"qps{uid[0]}_{n}")
                       for n in ns]
                for k in range(nkc):
                    lhsT = wqT_a[:, (m * nkc + k) * 128:(m * nkc + k + 1) * 128]
                    for ps, n in zip(pss, ns):
                        nc.tensor.matmul(
                            ps[:], lhsT=lhsT,
                            rhs=xT_a[:, k * T + n * 512:k * T + (n + 1) * 512],
                            start=(k == 0), stop=(k == nkc - 1),
                        )
                    yield 512 * len(ns)
                for ps, n in zip(pss, ns):
                    nc.vector.tensor_scalar_add(
                        qp_t[m][:, n * 512:(n + 1) * 512], ps[:],
                        bias_t[:, OQ + m:OQ + m + 1])
                q_done.update((m, n) for n in ns)
                yield 64

            OENG = [nc.sync, nc.gpsimd]
            oq = [0]

            o_state = {}

            def o_quanta(mt, tag="f", kts=None):
                """Output-projection rows mt*128: list of (kind, cost, fn)
                quanta for the pv queue (FIFO after the norms that write its
                yT). kts selects a subset of the accumulation steps so the
                final rows can pre-run all but their last yT dependency."""
                state = o_state.setdefault(mt, {})
                kts = range(nmq) if kts is None else kts
                last = max(kts) == nmq - 1

                def mm_q(kt):
                    def fn():
                        if "pss" not in state:
                            uid[0] += 1
                            state["pss"] = [
                                psum_tile([128, 512], tag, f"ops{uid[0]}_{h2}")
                                for h2 in range(2)]
                        lhsT = yT_t[kt][:, mt * 128:(mt + 1) * 128]
                        for h2 in range(2):
                            nc.tensor.matmul(
                                state["pss"][h2][:], lhsT=lhsT,
                                rhs=woT_a[:, kt * C + h2 * 512:
                                          kt * C + (h2 + 1) * 512],
                                start=(kt == 0), stop=(kt == nmq - 1),
                            )
                    return ("o", 1024, fn)

                def out_q(h2):
                    def fn():
                        uid[0] += 1
                        o_sb = wp.tile([128, 512], mybir.dt.float16, tag="o",
                                       name=f"osb{uid[0]}_{h2}", bufs=4)
                        nc.vector.tensor_copy(o_sb[:], state["pss"][h2][:])
                        eng = OENG[oq[0] % 2]
                        oq[0] += 1
                        eng.dma_start(
                            out=out_d[mt * 128:(mt + 1) * 128,
                                      h2 * 512:(h2 + 1) * 512],
                            in_=o_sb[:])
                    return ("o", 64, fn)

                qs = [mm_q(kt) for kt in kts]
                if last:
                    qs += [out_q(0), out_q(1)]
                return qs

            # ---- filler queue in need-order ----
            # k0 remainder chunks, split at the front boundary FR
            k0_rest = [(256, min(FR, tkp) - 256)] if tkp > 256 else []
            k0_rest += [(s0, min(512, tkp - s0)) for s0 in range(FR, tkp, 512)]
            k_chunks = [(s0, min(512, tkp - s0)) for s0 in range(0, tkp, 512)]
            k_groups = [k_chunks[i:i + 2] for i in range(0, len(k_chunks), 2)]

            # need-order: k0 remainder interleaved with the first V tiles
            # (scores t2+ need kT columns before PVs need va), q chunk 1
            # after ~6 V tiles, then per-pair k+q for the later attentions
            gens = []
            nv = 0
            for ch in k0_rest:
                gens.append(gen_k_group(0, [ch]))
                if nv < ntk:
                    gens.append(gen_v_tile(nv))
                    nv += 1
            while nv < min(6, ntk):
                gens.append(gen_v_tile(nv))
                nv += 1
            gens.append(gen_q_group(0, [1]))
            while nv < ntk:
                gens.append(gen_v_tile(nv))
                nv += 1
            for m in range(1, nmq):
                for g in k_groups:
                    gens.append(gen_k_group(m, g))
                gens.append(gen_q_group(m, [0, 1]))

            # emitted-work ledger: pump background work only while the PE
            # queue has less emitted work than the exp stream will take to
            # execute. Two sources: `gens` (projection fillers - they gate
            # future exps, so they go first) and `pvq` (the cross-attention
            # PV/norm/output FIFO - it only gates psum slots). `cool` spaces
            # the first PV after a norm so the normalize's DVE latency never
            # stalls the in-order PE queue.
            led = {"pe": 0, "exp": 0, "cool": 0}
            pvq = []          # FIFO of (kind, cost, fn)
            npv = [0]         # count of 'pv' items in pvq (p_sb slots held)

            def force_step():
                try:
                    led["pe"] += next(gens[0])
                except StopIteration:
                    gens.pop(0)

            def pop_pvq():
                kind, cost, fn = pvq.pop(0)
                if kind == "pv":
                    npv[0] -= 1
                if kind == "norm":
                    led["cool"] = led["pe"] + 4200
                fn()
                led["pe"] += cost

            def pump():
                while led["pe"] + 500 < led["exp"]:
                    if npv[0] > 12:
                        pop_pvq()
                    elif gens:
                        force_step()
                    elif pvq and (led["pe"] >= led["cool"]
                                  or pvq[0][0] != "pv"):
                        pop_pvq()
                    else:
                        break

            def drain_pvq():
                while pvq:
                    pop_pvq()

            def ensure_va(t):
                while t not in va_done and gens:
                    force_step()

            def ensure_kq(m, t_need, qc):
                while gens and (kcov[m] < t_need or (m, qc) not in q_done):
                    force_step()

            EXPF = mybir.ActivationFunctionType.Exp

            def attention(m, qc):
                """Head pair m (heads 2m, 2m+1), query chunk qc (512 wide).
                Emits only the S matmuls and exps; PV accumulation and the
                normalize are queued on pvq and drain under the ledger."""
                q0 = qc * 512
                uid[0] += 1
                yps = psum_tile([128, 512], "y", f"yps{uid[0]}")
                yps2 = psum_tile([128, 512], "y", f"yps2_{uid[0]}")

                def pv_fn(t, p_sb):
                    def fn():
                        ensure_va(t)
                        nc.tensor.matmul(
                            yps[:],
                            lhsT=va_t[t][:, (2 * m) * 128:(2 * m + 1) * 128],
                            rhs=p_sb[:, 0:512],
                            start=(t == 0), stop=(t == ntk - 1),
                        )
                        nc.tensor.matmul(
                            yps2[:],
                            lhsT=va_t[t][:, (2 * m + 1) * 128:(2 * m + 2) * 128],
                            rhs=p_sb[:, 512:1024],
                            start=(t == 0), stop=(t == ntk - 1),
                        )
                    return fn

                def norm_fn():
                    # normalize straight from psum: partitions 0-63 hold the
                    # denominator (ones-columns), 64-127 the numerator
                    def fn():
                        uid[0] += 1
                        for hp, ps in ((0, yps), (1, yps2)):
                            rec = wp.tile([128, 512], F32, tag="rec",
                                          name=f"rec{uid[0]}_{hp}", bufs=2)
                            nc.vector.reciprocal_approx_fast(
                                rec[0:64, :], ps[0:64, :])
                            nc.vector.tensor_mul(
                                yT_t[m][64 * hp:64 * hp + 64, q0:q0 + 512],
                                ps[64:128, :], rec[0:64, :],
                            )
                    return fn

                def s_mm(t):
                    # two K=64 matmuls on independent 64-row PE tiles
                    uid[0] += 1
                    s_ps = psum_tile([128, 1024], "s", f"sps{uid[0]}")
                    nc.tensor.matmul(
                        s_ps[:, 0:512],
                        lhsT=kT_t[m][0:64, t * 128:(t + 1) * 128],
                        rhs=qp_t[m][0:64, q0:q0 + 512],
                        start=True, stop=True,
                    )
                    nc.tensor.matmul(
                        s_ps[:, 512:1024],
                        lhsT=kT_t[m][64:128, t * 128:(t + 1) * 128],
                        rhs=qp_t[m][64:128, q0:q0 + 512],
                        start=True, stop=True,
                    )
                    led["pe"] += 512
                    return s_ps

                # S matmuls batched two t-steps at a time: the 64-row loads of
                # step t+1 overlap the streaming of step t's opposite tile
                for t0 in range(0, ntk, 2):
                    ts = [t for t in (t0, t0 + 1) if t < ntk]
                    ensure_kq(m, (ts[-1] + 1) * 128, qc)
                    sps = [s_mm(t) for t in ts]
                    for t, s_ps in zip(ts, sps):
                        p_sb = wp.tile([128, 1024], BF16, tag="p",
                                       name=f"p{uid[0]}_{t}", bufs=20)
                        nc.scalar.activation(
                            p_sb[:], s_ps[:], EXPF,
                            bias=bias_t[:, OM + t:OM + t + 1], scale=float(SCALE),
                        )
                        led["exp"] += EXP_CYC
                        pvq.append(("pv", 1024, pv_fn(t, p_sb)))
                        npv[0] += 1
                    pump()
                pvq.append(("norm", 64, norm_fn()))

            # schedule: qc 0/1 interleaved per pair (halves the prerequisite
            # pressure of each pair's k+q projections), then qc 2/3 pair-major
            sched = ([(m, qc) for m in range(NP) for qc in (0, 1)]
                     + [(m, qc) for qc in (2, 3) for m in range(NP)])
            for i, (m, qc) in enumerate(sched):
                if i == 5:
                    for mm in range(nmq):
                        gens.append(gen_q_group(mm, [2, 3]))
                if i == 8:
                    # qc 0/1 norms are already queued ahead in the FIFO
                    for mt in range(0, 8):
                        pvq.extend(o_quanta(mt))
                if i == 12:
                    for mt in range(8, 12):
                        pvq.extend(o_quanta(mt))
                if i == 15:
                    # final rows 12/13: all but the last accumulation step can
                    # run during the last attention (their yT is ready)
                    for mt in (12, 13):
                        pvq.extend(o_quanta(mt, kts=[0, 1, 2]))
                attention(m, qc)

            # drain: remaining projections, the PV/norm backlog, then the
            # final output rows. Rows 14/15 accumulate their first steps on
            # the s slots (free once the last exp has read them) while the
            # final normalize runs on the vector engine.
            led["exp"] += 1 << 30
            while gens:
                force_step()
            for mt in (14, 15):
                pvq.extend(o_quanta(mt, "s", kts=[0, 1, 2]))
            for mt in (12, 13, 14, 15):
                pvq.extend(o_quanta(mt, kts=[3]))
            drain_pvq()

    _dedup_ldweights(nc)
    nc.compile()
    return nc


def _get_nc(tkp: int):
    if tkp not in _nc_cache:
        _nc_cache[tkp] = _build_nc(tkp)
    return _nc_cache[tkp]


def kernel(x, mask, Wk, bk, Wq, bq, Wv, bv, Wo, bo, _run_kwargs=None):
    x = np.asarray(x, dtype=np.float32)
    mask = np.asarray(mask)
    Wk, bk = np.asarray(Wk, np.float32), np.asarray(bk, np.float32)
    Wq, bq = np.asarray(Wq, np.float32), np.asarray(bq, np.float32)
    Wv, bv = np.asarray(Wv, np.float32), np.asarray(bv, np.float32)
    Wo, bo = np.asarray(Wo, np.float32), np.asarray(bo, np.float32)

    keep = [np.flatnonzero(mask[b] == 0) for b in range(B)]
    max_keep = max(len(kp) for kp in keep)
    tkp = max(128, -(-max_keep // 128) * 128)
    ntk = tkp // 128
    nmq = DL // 128

    nc = _get_nc(tkp)

    in_maps = []
    orders = []
    for b in range(B):
        # kept-key positions first: the device reads keys as xT[:, :tkp]
        order = np.concatenate([keep[b], np.flatnonzero(mask[b] != 0)])
        orders.append(order)
        xT = np.ascontiguousarray(x[b][order].T).astype(NP_BF16)
        mb = np.zeros(tkp, np.float32)
        mb[len(keep[b]):] = NEG
        mbp = np.ascontiguousarray(mb.reshape(ntk, 128).T)
        for g in range(G):
            gs, ge = g * DL, (g + 1) * DL
            bias_all = np.concatenate([
                bq[gs:ge].reshape(nmq, 128).T,
                bk[gs:ge].reshape(nmq, 128).T,
                np.broadcast_to(bv[gs:ge], (128, DL)),
                mbp,
            ], axis=1).astype(np.float32)

            def mswz(W):
                # [C, DL] -> [128, m-block x k-chunk x 128]: block (m, k)
                # holds W.T rows k*128:(k+1)*128, cols m*128:(m+1)*128
                A = np.ascontiguousarray(W[gs:ge].T).reshape(8, 128, nmq, 128)
                return np.ascontiguousarray(
                    A.transpose(1, 2, 0, 3).reshape(128, nmq * 8 * 128)
                ).astype(NP_BF16)

            in_maps.append({
                "xT": xT,
                "wqT": mswz(Wq),
                "wkT": mswz(Wk),
                "wvT": np.ascontiguousarray(Wv[gs:ge].T).astype(NP_BF16),
                "woT": np.ascontiguousarray(Wo[:, gs:ge].T).astype(NP_BF16),
                "bias_all": np.ascontiguousarray(bias_all),
            })

    kw = _run_kwargs or {}
    res = run_bass_kernel_spmd(nc, in_maps, list(range(N_CORES)), **kw)

    out = np.empty((B, T, C), np.float32)
    for b in range(B):
        summed = (res.results[2 * b]["out"].astype(np.float32)
                  + res.results[2 * b + 1]["out"].astype(np.float32) + bo)
        out[b][orders[b]] = summed  # undo the query-position permutation
    if kw:
        kernel.last_result = res
    return out
